# revision 71
# baseline (speedup 1.0000x reference)
"""Local (windowed) attention kernel for TRN2, 8 NeuronCores, SPMD. v2.

Reference computation (B=4, N=8192, DIM=1024, H=16, DH=64, W=128):
    q = x @ wq ; k,v = split(x @ wkv)
    per (batch, head, window of 128): attend to [prev window, cur window]
    with causal mask (j > i + W masked), softmax, out = attn @ v
    out = out @ wo + bo

Sharding: sequence dim split into 8 contiguous chunks of 1024 tokens, one
per core, each with a 128-token front halo (zeros for core 0 — matches the
reference's zero-pad).  Weights replicated; no collectives.

v3 design (vs v2):
  * scores MMs interleave the two 64-partition row groups (e=0/e=1) so
    adjacent matmuls sit on disjoint PE row groups and run concurrently
    (v2 emitted all of e=0 then all of e=1, serializing LDW+MM per head).
  * softmax denominators: the stationary ones-vector is [128, 64] instead
    of [128, 1], so the sums matmul itself replicates the denominator row
    across 64 output partitions (M is free on the PE).  The two (p) streams
    land at psum partitions 0-63 / 64-127 of one bank per head-group (2-way
    col-tiled, concurrent).  This kills the v2 GpSimd partition_broadcast
    (141 us) and the skinny [1, 1024] reciprocals (77 us): one DVE
    reciprocal per [128, 512] sums bank feeds the normalize muls directly.
  * normalize fused into PSUM evacuation: one DVE tensor_mul per
    (head-parity, 4-head group) writing normalized bf16 straight into the
    feature-major attention output aoT.
  * output DRAM tensor is bf16 (host converts to f32) to halve store DMA.
"""

import numpy as np
import ml_dtypes

import concourse.bass as bass
import concourse.bacc as bacc
import concourse.mybir as mybir
import concourse.tile as tile
from concourse.bass_utils import run_bass_kernel_spmd

B, N, DIM = 4, 8192, 1024
H, DH, W = 16, 64, 128
NCORES = 8
TOW = N // NCORES          # own tokens per core per batch   = 1024
TH = TOW + W               # with front halo                 = 1152
NW = TOW // W              # query windows per core-batch    = 8
NKW = NW + 1               # key windows incl. halo          = 9
KT = DIM // 128            # contraction tiles               = 8
MT = DIM // 128            # inner/output tiles              = 8
SCALE = DH ** -0.5

BF16 = mybir.dt.bfloat16
F32 = mybir.dt.float32
AF = mybir.ActivationFunctionType

TRACE = False              # set by test.py to collect an NTFF profile
TRACE_KW = {}
LAST_RESULT = None         # BassKernelResults stash when TRACE
REPEAT = 1                 # whole-computation repeats inside the NEFF (bench)


def _build_bass():
    nc = bacc.Bacc(None, target_bir_lowering=False)
    xT = nc.declare_dram_parameter("xT", [B, DIM, TH], BF16, isOutput=False)
    wq = nc.declare_dram_parameter("wq", [DIM, DIM], BF16, isOutput=False)
    wkv = nc.declare_dram_parameter("wkv", [DIM, 2 * DIM], BF16, isOutput=False)
    wo = nc.declare_dram_parameter("wo", [DIM, DIM], BF16, isOutput=False)
    bo_pm = nc.declare_dram_parameter("bo_pm", [128, MT], F32, isOutput=False)
    maskU = nc.declare_dram_parameter("maskU", [128, 128], BF16, isOutput=False)
    outT = nc.declare_dram_parameter("outT", [B, DIM, TOW], BF16, isOutput=True)

    with tile.TileContext(nc) as tc:
        with (
            tc.tile_pool(name="wpool", bufs=1) as wpool,
            tc.tile_pool(name="xpool", bufs=1) as xpool,
            tc.tile_pool(name="qpool", bufs=2) as qpool,
            tc.tile_pool(name="actpool", bufs=1) as actpool,
            tc.tile_pool(name="apool", bufs=4) as apool,
            tc.tile_pool(name="rpool", bufs=3) as rpool,
            tc.tile_pool(name="opool", bufs=2) as opool,
            tc.tile_pool(name="pbig", bufs=4, space="PSUM") as pbig,
            tc.tile_pool(name="ppv", bufs=2, space="PSUM") as ppv,
            tc.tile_pool(name="prows", bufs=2, space="PSUM") as prows,
        ):
            # ---- replicated constants (loaded once) ----
            wq_sb = wpool.tile([128, KT, DIM], BF16)
            wkv_sb = wpool.tile([128, KT, 2 * DIM], BF16)
            wo_sb = wpool.tile([128, KT, DIM], BF16)
            bo_sb = wpool.tile([128, MT], F32)
            mask_sb = wpool.tile([128, 4, 128], BF16)
            ones_sb = wpool.tile([128, 64], BF16)
            # wq first, split across the sync+scalar queues (the first
            # q-proj chain needs all of it); wkv/wo follow behind the
            # batch-0 x chunks (k-proj starts ~40us in, so wkv lands in
            # time).
            for k in range(KT):
                eng = nc.sync if k < 4 else nc.scalar
                eng.dma_start(out=wq_sb[:, k, :], in_=wq[k * 128:(k + 1) * 128, :])
            nc.vector.memset(ones_sb, 1.0)

            def _load_x(bload, engs):
                # chunked per k-tile across DMA queues: the first chunk
                # lands much earlier than one monolithic transfer.
                x_t = xpool.tile([128, KT, TH], BF16, tag="x", name="x_t")
                for k in range(KT):
                    engs[k % len(engs)].dma_start(
                        out=x_t[:, k, :],
                        in_=xT[bload, k * 128:(k + 1) * 128, :],
                    )
                return x_t

            def _load_x0(bload):
                # batch 0: balance BYTES across the three idle queues
                # (sync/scalar also carry 1 MB of wq each): gpsimd gets 4
                # chunks, sync/scalar 2 each.
                x_t = xpool.tile([128, KT, TH], BF16, tag="x", name="x_t")
                x0_engs = [nc.gpsimd, nc.sync, nc.scalar, nc.gpsimd,
                           nc.gpsimd, nc.sync, nc.gpsimd, nc.scalar]
                for k in range(KT):
                    x0_engs[k].dma_start(
                        out=x_t[:, k, :],
                        in_=xT[bload, k * 128:(k + 1) * 128, :],
                    )
                return x_t

            def _emit_qproj(x_t, qT_t, chains):
                # q projection, feature-major: qT[m] = wq[:,m].T @ x
                # (evacuation on DVE: ScalarE is busy with exp when these
                # chains run inside the previous batch's attention loop)
                for m, c in chains:
                    ps = pbig.tile([128, 512], F32, tag="big", name="ps_q")
                    for k in range(KT):
                        nc.tensor.matmul(
                            ps,
                            lhsT=wq_sb[:, k, m * 128:(m + 1) * 128],
                            rhs=x_t[:, k, W + c * 512:W + (c + 1) * 512],
                            start=(k == 0),
                            stop=(k == KT - 1),
                        )
                    nc.vector.tensor_copy(
                        qT_t[:, m, c * 512:(c + 1) * 512], ps,
                    )

            QCHAINS = [(m, c) for m in range(MT) for c in range(2)]
            # qproj(b+1) chains dripped into batch b's attention tail,
            # keyed by kw iteration; the remainder is emitted just before
            # out-proj c=1 so the scheduler interleaves the two.
            QDRIP = {5: QCHAINS[0:2], 6: QCHAINS[2:4], 7: QCHAINS[4:7],
                     8: QCHAINS[7:10]}
            QREST = QCHAINS[10:16]

            blist = [bb % B for bb in range(B * REPEAT)]
            # batch 0 prologue: x split across three idle queues, then the
            # remaining weight loads queue behind its sync chunks, then the
            # whole q projection.
            x_cur = _load_x0(blist[0])
            for k in range(KT):
                nc.sync.dma_start(out=wkv_sb[:, k, :],
                                  in_=wkv[k * 128:(k + 1) * 128, :])
                nc.sync.dma_start(out=wo_sb[:, k, :],
                                  in_=wo[k * 128:(k + 1) * 128, :])
            nc.sync.dma_start(out=bo_sb, in_=bo_pm[:])
            for mc in range(4):
                nc.sync.dma_start(out=mask_sb[:, mc, :], in_=maskU[:])
            qT = qpool.tile([128, MT, TOW], BF16, tag="qT")
            _emit_qproj(x_cur, qT, QCHAINS)

            for bi, b in enumerate(blist):
                x_sb = x_cur
                b_next = blist[bi + 1] if bi + 1 < len(blist) else None
                kTt = actpool.tile([128, MT, TH], BF16, tag="kT")
                v_sb = actpool.tile([128, NKW, DIM], BF16, tag="v")
                aoT = actpool.tile([128, MT, TOW], BF16, tag="aoT")

                # ---- k projection, feature-major (incl. halo) ----
                for m in range(MT):
                    for c in range(3):
                        ps = pbig.tile([128, 512], F32, tag="big")
                        for k in range(KT):
                            nc.tensor.matmul(
                                ps[:, 0:384],
                                lhsT=wkv_sb[:, k, m * 128:(m + 1) * 128],
                                rhs=x_sb[:, k, c * 384:(c + 1) * 384],
                                start=(k == 0),
                                stop=(k == KT - 1),
                            )
                        nc.scalar.activation(
                            out=kTt[:, m, c * 384:(c + 1) * 384],
                            in_=ps[:, 0:384],
                            func=AF.Copy,
                            bias=0.0,
                            scale=1.0,
                        )

                # ---- v projection, token-major per key-window ----
                for kw in range(NKW):
                    for c in range(2):
                        ps = pbig.tile([128, 512], F32, tag="big")
                        for k in range(KT):
                            nc.tensor.matmul(
                                ps,
                                lhsT=x_sb[:, k, kw * 128:(kw + 1) * 128],
                                rhs=wkv_sb[:, k, DIM + c * 512:DIM + (c + 1) * 512],
                                start=(k == 0),
                                stop=(k == KT - 1),
                            )
                        nc.vector.tensor_copy(
                            v_sb[:, kw, c * 512:(c + 1) * 512], ps,
                        )

                # ---- attention: key-window-major transposed scores ----
                _last_b = b_next is None

                def _emit_out_proj(c, ms=range(MT), _b=b, _aoT=aoT,
                                   _last=_last_b):
                    for m in ms:
                        ps = pbig.tile([128, 512], F32, tag="big", name="ps_op")
                        for k in range(KT):
                            nc.tensor.matmul(
                                ps,
                                lhsT=wo_sb[:, k, m * 128:(m + 1) * 128],
                                rhs=_aoT[:, k, c * 512:(c + 1) * 512],
                                start=(k == 0),
                                stop=(k == KT - 1),
                            )
                        osb = opool.tile([128, 512], BF16, tag="outsb")
                        nc.scalar.add(osb, ps, bo_sb[:, m:m + 1])
                        # last batch: gpsimd is free (no more x loads), so
                        # split its tail stores across two queues; other
                        # batches stay on sync (gpsimd stores would delay
                        # the x chunks feeding the q-proj drip).
                        seng = nc.gpsimd if (_last and m % 2 == 1) else nc.sync
                        seng.dma_start(
                            out=outT[_b, m * 128:(m + 1) * 128,
                                     c * 512:(c + 1) * 512],
                            in_=osb,
                        )

                attn_tiles = []
                x_next = qT_next = None
                for kw in range(NKW):
                    if kw == 0 and b_next is not None:
                        # next batch's x load + qT buffer: the x DMAs fire
                        # once k/v-proj(b) release x_sb, landing mid-
                        # attention; qproj(b+1) chains drip in below.
                        x_next = _load_x(b_next, [nc.gpsimd, nc.sync])
                        qT_next = qpool.tile([128, MT, TOW], BF16,
                                             tag="qT", name="qT_next")
                    has_a = kw >= 1          # cur-block for qw = kw-1
                    has_b = kw <= NW - 1     # prev-block for qw = kw
                    ncols = 128 * (has_a + has_b)
                    # qT is indexed by own tokens (haloed minus W)
                    qcol0 = (kw - 1) * 128 if has_a else 0
                    at = apool.tile([128, H, 256], BF16, tag="attn")
                    attn_tiles.append(at)
                    # attn tile slot permutation: slot(h) = (h%2)*8 + h//2.
                    # Score MMs alternate the two 64-partition row groups
                    # (e=0/e=1) so adjacent MMs occupy disjoint PE row
                    # groups and run concurrently; the two mi MMs of one
                    # row group stay serial among themselves, keeping the
                    # same-bank psum drains safe.
                    for mp in range(0, MT, 2):
                        scs = [
                            pbig.tile([128, 512], F32, tag="big",
                                      name=f"sc_e{e}")
                            for e in range(2)
                        ]
                        for mi in range(2):
                            for e in range(2):
                                r = e * 64
                                nc.tensor.matmul(
                                    scs[e][:, mi * 256:mi * 256 + ncols],
                                    lhsT=kTt[r:r + 64, mp + mi,
                                             kw * 128:(kw + 1) * 128],
                                    rhs=qT[r:r + 64, mp + mi,
                                           qcol0:qcol0 + ncols],
                                    start=True,
                                    stop=True,
                                    skip_group_check=True,
                                )
                        for e in range(2):
                            sl = e * 8 + mp
                            nc.scalar.activation(
                                out=at[:, sl:sl + 2, 0:ncols],
                                in_=scs[e][:].rearrange(
                                    "p (h i) -> p h i", i=256
                                )[:, :, 0:ncols],
                                func=AF.Exp,
                                bias=0.0,
                                scale=SCALE,
                            )
                            if has_a and mp % 4 == 2:
                                # multiplicative causal mask on the cur
                                # block: at[j, i] *= (j <= i).  One mul per
                                # 4 adjacent slots (this + previous
                                # mp-group of the same row-group e).
                                slb4 = e * 8 + mp - 2
                                nc.vector.tensor_mul(
                                    out=at[:, slb4:slb4 + 4, 0:128],
                                    in0=at[:, slb4:slb4 + 4, 0:128],
                                    in1=mask_sb,
                                )

                    # ---- pv + sums + normalize for qw = kw-1 ----
                    if kw == 0:
                        continue
                    qw = kw - 1
                    at_prev = attn_tiles[qw]       # tile kw-1: B block
                    at_cur = attn_tiles[kw]        # tile kw:   A block
                    bcol = 0 if qw == 0 else 128
                    for c in range(2):
                        pvt = ppv.tile([128, 512], F32, tag="pv")
                        srow = prows.tile([128, 512], F32, tag="srows")
                        pv_last = None
                        for hh in range(8):
                            h = 8 * c + hh
                            s, p = hh // 2, hh % 2
                            sl = (h % 2) * 8 + h // 2
                            hv = slice(h * 64, (h + 1) * 64)
                            pv_out = pvt[p * 64:p * 64 + 64, s * 128:(s + 1) * 128]
                            nc.tensor.matmul(
                                pv_out,
                                lhsT=v_sb[:, qw, hv],
                                rhs=at_prev[:, sl, bcol:bcol + 128],
                                start=True,
                                stop=True,
                            )
                            pv_last = nc.tensor.matmul(
                                pv_out,
                                lhsT=v_sb[:, kw, hv],
                                rhs=at_cur[:, sl, 0:128],
                                start=False,
                                stop=False,
                                skip_group_check=True,
                            )
                        # denominators: ones [128, 64] stationary makes the
                        # PE replicate the sums row to 64 partitions; the
                        # two p streams col-tile to partition halves 0/64
                        # of one bank and run concurrently.
                        srow_lasts = []
                        for p in range(2):
                            slb = p * 8 + 4 * c
                            nc.tensor.matmul(
                                srow[p * 64:(p + 1) * 64, :],
                                lhsT=ones_sb[:, 0:64],
                                rhs=at_prev[:, slb:slb + 4, bcol:bcol + 128],
                                start=True,
                                stop=True,
                                skip_group_check=True,
                            )
                        for p in range(2):
                            slb = p * 8 + 4 * c
                            srow_lasts.append(nc.tensor.matmul(
                                srow[p * 64:(p + 1) * 64, :],
                                lhsT=ones_sb[:, 0:64],
                                rhs=at_cur[:, slb:slb + 4, 0:128],
                                start=False,
                                stop=False,
                                skip_group_check=True,
                            ))
                        rr = rpool.tile([128, 512], F32, tag="recip")
                        ri = nc.vector.reciprocal_approx_fast(
                            out=rr, in_=srow
                        )
                        for sv in srow_lasts:
                            tile.add_dep_helper(ri.ins, sv.ins)
                        # pvt rows, rr rows and aoT rows are all aligned:
                        # one mul covers both parity halves.
                        mi = nc.vector.tensor_mul(
                            out=aoT[:, 4 * c:4 * c + 4,
                                    qw * 128:(qw + 1) * 128],
                            in0=pvt[:].rearrange("p (s i) -> p s i", i=128),
                            in1=rr[:].rearrange("p (s i) -> p s i", i=128),
                        )
                        tile.add_dep_helper(mi.ins, pv_last.ins)

                    # drip exp-independent PE work into the attention
                    # iterations so the in-order PE queue has something to
                    # run where the attention chain stalls on ScalarE:
                    # qproj(b+1) chains once x(b+1) has landed, and
                    # out-proj c=0 (needs aoT cols 0:512 = qw 0-3, done
                    # after the kw=4 iteration).
                    if b_next is not None and kw in QDRIP:
                        _emit_qproj(x_next, qT_next, QDRIP[kw])
                    if 5 <= kw <= 8:
                        mlo = 2 * (kw - 5)
                        _emit_out_proj(0, ms=(mlo, mlo + 1))

                if b_next is not None:
                    _emit_qproj(x_next, qT_next, QREST)
                _emit_out_proj(1)
                x_cur = x_next
                qT = qT_next
    nc.compile()
    return nc


_NC_CACHE = None


def _get_nc():
    global _NC_CACHE
    if _NC_CACHE is None:
        _NC_CACHE = _build_bass()
    return _NC_CACHE


def kernel(x, wq, wkv, wo, bo):
    global LAST_RESULT
    bfd = ml_dtypes.bfloat16
    x = np.asarray(x, np.float32)
    wq_b = np.asarray(wq, np.float32).astype(bfd)
    wkv_b = np.asarray(wkv, np.float32).astype(bfd)
    wo_b = np.asarray(wo, np.float32).astype(bfd)
    bo_pm = np.ascontiguousarray(
        np.asarray(bo, np.float32).reshape(MT, 128).T
    )
    # maskU[j, i] = 0 where cur-window key j > query i (causal), else 1
    maskU = np.where(
        np.arange(W)[:, None] > np.arange(W)[None, :], 0.0, 1.0
    ).astype(bfd)

    xb = x.astype(bfd)
    in_maps = []
    for c in range(NCORES):
        lo, hi = c * TOW - W, (c + 1) * TOW
        if c == 0:
            sl = np.concatenate(
                [np.zeros((B, W, DIM), bfd), xb[:, :hi]], axis=1
            )
        else:
            sl = xb[:, lo:hi]
        xT_c = np.ascontiguousarray(sl.transpose(0, 2, 1))  # [B, DIM, TH]
        in_maps.append(
            dict(xT=xT_c, wq=wq_b, wkv=wkv_b, wo=wo_b, bo_pm=bo_pm,
                 maskU=maskU)
        )

    nc = _get_nc()
    res = run_bass_kernel_spmd(
        nc, in_maps, list(range(NCORES)), trace=TRACE, **TRACE_KW
    )
    if TRACE:
        LAST_RESULT = res
    out = np.empty((B, N, DIM), np.float32)
    for c in range(NCORES):
        out[:, c * TOW:(c + 1) * TOW, :] = (
            res.results[c]["outT"].astype(np.float32).transpose(0, 2, 1)
        )
    return out

